# revision 1
# baseline (speedup 1.0000x reference)
"""Trainium2 Bass kernel for windowed (sparse) attention.

Module: LayerNorm -> overlapping 8x8 spatial windows (stride 6) over a
[2,2,128,128,256] image -> per-window 8-head attention over L=128 tokens
(t*8*8) -> output projection -> overlap-add with count normalization.

Strategy: 882 independent windows sharded over 8 cores (111 each, padded
to 888).  Host does im2win gather + overlap-add scatter (pure data
movement); all model compute (LN, QKV, attention, Wo) runs on device.

Device dataflow per window (all matmuls fp16 in / fp32 psum):
  x[128,256] --LN(DVE)--> xn fp16 --DMA-transpose--> xnT[c,l]
  qT/kT = W.T-stationary matmuls        (weights pre-transposed on host)
  v     = xnT-stationary matmul          -> V[l(token),mid] natural
  S^T_h = kT_h.T @ qT_h  (K=32, 4 heads row-packed via tile_position)
  E^T   = exp(S^T / sqrt(32))            (one ACT op over all 8 heads)
  D_h   = ones.T @ E^T_h  (col-packed)   = softmax denominators
  Dinv  = reciprocal_approx(D); Dinvb = indicator-matmul broadcast
  OT_u  = V_h-stationary @ E^T_h (col-packed), OT = OT_u * Dinvb (fused evac)
  ZT    = WoT-stationary @ OT            -> out DRAM (host adds bo)
"""

import functools
import math
from contextlib import ExitStack

import numpy as np

import concourse.bacc as bacc
import concourse.bass as bass
import concourse.mybir as mybir
import concourse.tile as tile
from concourse.bass_utils import run_bass_kernel_spmd

# Problem constants (hardcoded per contract - kernel.py is self-contained).
B, T, H, W, C = 2, 2, 128, 128, 256
MID, HEADS = 256, 8
HD = MID // HEADS          # 32
PATCH, STEP = 8, 6         # window size / stride
NHW = 21                   # windows per axis: starts 0,6,...,120
NWIN = NHW * NHW * B       # 882 flat windows (n outer, b inner)
L = T * PATCH * PATCH      # 128 tokens per window
NCORES = 8
NW = 111                   # windows per core after padding to 888
EPS = 1e-6
F32, F16, F32R = mybir.dt.float32, mybir.dt.float16, mybir.dt.float32r
AF = mybir.ActivationFunctionType
ALU = mybir.AluOpType


def _build_program(nw: int):
    nc = bacc.Bacc(
        "TRN2",
        target_bir_lowering=False,
        debug=False,
        enable_asserts=False,
        num_devices=NCORES,
    )
    xw = nc.dram_tensor("xw", [nw * 128, 256], F32, kind="ExternalInput").ap()
    wq = nc.dram_tensor("wq", [256, 256], F16, kind="ExternalInput").ap()
    wk = nc.dram_tensor("wk", [256, 256], F16, kind="ExternalInput").ap()
    wv = nc.dram_tensor("wv", [256, 256], F16, kind="ExternalInput").ap()
    wo = nc.dram_tensor("wo", [256, 256], F16, kind="ExternalInput").ap()
    ones1 = nc.dram_tensor("ones1", [128, 32], F16, kind="ExternalInput").ap()
    zt = nc.dram_tensor("zt", [nw * 256, 128], F32, kind="ExternalOutput").ap()

    inv_sqrt_hd = 1.0 / math.sqrt(HD)

    with tile.TileContext(nc) as tc, ExitStack() as ctx:
        pw = ctx.enter_context(tc.tile_pool(name="wts", bufs=1))
        # Persistent weight tiles: chunk kc holds input-dim rows kc*128..+128.
        wq_s = [pw.tile([128, 256], F16, tag=f"wq{i}", name=f"wq{i}") for i in range(2)]
        wk_s = [pw.tile([128, 256], F16, tag=f"wk{i}", name=f"wk{i}") for i in range(2)]
        wv_s = [pw.tile([128, 256], F16, tag=f"wv{i}", name=f"wv{i}") for i in range(2)]
        wo_s = [pw.tile([128, 256], F16, tag=f"wo{i}", name=f"wo{i}") for i in range(2)]
        for i in range(2):
            nc.sync.dma_start(wq_s[i][:], wq[i * 128:(i + 1) * 128, :])
            nc.sync.dma_start(wk_s[i][:], wk[i * 128:(i + 1) * 128, :])
            nc.sync.dma_start(wv_s[i][:], wv[i * 128:(i + 1) * 128, :])
            nc.sync.dma_start(wo_s[i][:], wo[i * 128:(i + 1) * 128, :])
        ones_s = pw.tile([128, 32], F16, tag="ones1")
        nc.sync.dma_start(ones_s[:], ones1[:])

        # SBUF pools
        px = ctx.enter_context(tc.tile_pool(name="px", bufs=3))
        psc = ctx.enter_context(tc.tile_pool(name="psc", bufs=2))
        pst = ctx.enter_context(tc.tile_pool(name="pst", bufs=3))
        pxn = ctx.enter_context(tc.tile_pool(name="pxn", bufs=3))
        pxnt = ctx.enter_context(tc.tile_pool(name="pxnt", bufs=3))
        pqks = ctx.enter_context(tc.tile_pool(name="pqks", bufs=2))
        pqh = ctx.enter_context(tc.tile_pool(name="pqh", bufs=2))
        pvs = ctx.enter_context(tc.tile_pool(name="pvs", bufs=2))
        pes = ctx.enter_context(tc.tile_pool(name="pes", bufs=2))
        pdbs = ctx.enter_context(tc.tile_pool(name="pdbs", bufs=2))
        pos = ctx.enter_context(tc.tile_pool(name="pos", bufs=2))
        pzs = ctx.enter_context(tc.tile_pool(name="pzs", bufs=2))
        # PSUM pools: exactly 8 banks total.
        pqk = ctx.enter_context(tc.tile_pool(name="pqk", bufs=2, space="PSUM"))
        pv = ctx.enter_context(tc.tile_pool(name="pv", bufs=2, space="PSUM"))
        psp = ctx.enter_context(tc.tile_pool(name="psp", bufs=1, space="PSUM"))
        pmisc = ctx.enter_context(tc.tile_pool(name="pmisc", bufs=2, space="PSUM"))

        for w in range(nw):
            # ---- load + LayerNorm (stats in fp32 on DVE) ----
            xt = px.tile([128, 256], F32, tag="x")
            nc.sync.dma_start(xt[:], xw[w * 128:(w + 1) * 128, :])
            sx = pst.tile([128, 1], F32, tag="sx")
            nc.vector.reduce_sum(sx[:], xt[:], axis=mybir.AxisListType.X)
            mu = pst.tile([128, 1], F32, tag="mu")
            nc.vector.tensor_scalar_mul(mu[:], sx[:], 1.0 / 256)
            sq = psc.tile([128, 256], F32, tag="sq")  # discarded elementwise out
            vr = pst.tile([128, 1], F32, tag="vr")
            nc.vector.scalar_tensor_tensor(
                out=sq[:], in0=xt[:], scalar=mu[:], op0=ALU.subtract,
                op1=ALU.mult, in1=xt[:], accum_out=vr[:],
            )  # vr = sum((x-mu)*x) = 256*var   (tensor_tensor_reduce crashes HW)
            var = pst.tile([128, 1], F32, tag="var")
            nc.vector.tensor_scalar(
                out=var[:], in0=vr[:], scalar1=1.0 / 256, scalar2=EPS,
                op0=ALU.mult, op1=ALU.add,
            )
            std = pst.tile([128, 1], F32, tag="std")
            nc.scalar.activation(std[:], var[:], AF.Sqrt, bias=0.0)
            rs = pst.tile([128, 1], F32, tag="rs")
            nc.vector.reciprocal(rs[:], std[:])
            xn = pxn.tile([128, 256], F16, tag="xn")
            nc.vector.tensor_scalar(
                out=xn[:], in0=xt[:], scalar1=mu[:], scalar2=rs[:],
                op0=ALU.subtract, op1=ALU.mult,
            )
            # transpose xn -> xnT (two 128x128 fp16 DMA transposes)
            xnt = pxnt.tile([128, 256], F16, tag="xnt")
            nc.sync.dma_start_transpose(xnt[:, 0:128], xn[:, 0:128])
            nc.sync.dma_start_transpose(xnt[:, 128:256], xn[:, 128:256])

            # ---- q/k/v projections ----
            # One PSUM bank holds q|k; one zero-region group: start on the
            # first matmul in the bank, stop on the last.
            qkp = pqk.tile([128, 512], F32, tag="qk")
            first = True
            for ws, base in ((wq_s, 0), (wk_s, 256)):
                for mh in range(2):
                    for kc in range(2):
                        nc.tensor.matmul(
                            qkp[:, base + mh * 128:base + (mh + 1) * 128],
                            lhsT=ws[kc][:, mh * 128:(mh + 1) * 128],
                            rhs=xnt[:, kc * 128:(kc + 1) * 128],
                            start=first,
                            stop=(base == 256 and mh == 1 and kc == 1),
                        )
                        first = False
            vp = pv.tile([128, 512], F32, tag="v")
            for kc in range(2):
                nc.tensor.matmul(
                    vp[:, 0:256], lhsT=xnt[:, kc * 128:(kc + 1) * 128], rhs=wv_s[kc][:],
                    start=(kc == 0), stop=(kc == 1),
                )
            qks = pqks.tile([128, 512], F16, tag="qks")
            nc.scalar.activation(qks[:], qkp[:], AF.Copy)
            vs = pvs.tile([128, 256], F16, tag="vs")
            nc.vector.tensor_copy(vs[:], vp[:, 0:256])
            # Rearrange q/k head-major ([32, 8*128]) so every S matmul reads
            # operands at partition base 0 (row-group tile_position crashes
            # this runtime; base 96 is rejected at build).
            qh = pqh.tile([32, 1024], F16, tag="qh")
            kh = pqh.tile([32, 1024], F16, tag="kh")
            for h in range(HEADS):
                mh, hh = h // 4, h % 4
                nc.sync.dma_start(
                    qh[0:32, h * 128:(h + 1) * 128],
                    qks[32 * hh:32 * hh + 32, mh * 128:(mh + 1) * 128])
                nc.sync.dma_start(
                    kh[0:32, h * 128:(h + 1) * 128],
                    qks[32 * hh:32 * hh + 32, 256 + mh * 128:256 + (mh + 1) * 128])

            # ---- S^T = K Q^T per head (K=32 at base 0, unpacked) ----
            # sp spans 2 banks (4 heads each); per bank: start on first head,
            # stop on last.
            sp = psp.tile([128, 1024], F32, tag="sp")
            for h in range(HEADS):
                bankpos = h % 4
                nc.tensor.matmul(
                    sp[:, h * 128:(h + 1) * 128],
                    lhsT=kh[0:32, h * 128:(h + 1) * 128],
                    rhs=qh[0:32, h * 128:(h + 1) * 128],
                    start=(bankpos == 0), stop=(bankpos == 3),
                )
            es = pes.tile([128, 1024], F16, tag="es")
            nc.scalar.activation(es[:], sp[:], AF.Exp, scale=inv_sqrt_hd)

            # ---- softmax denominators: D_h[l] = sum_m E^T_h[m,l] ----
            # D rows live at partitions {0,32,64,96}; per partition the two
            # column rounds share a zero region: start on r=0, stop on r=1.
            # ones[128,32] stationary -> 32 identical D rows per head stripe:
            # dp[p, r*128+l] = D_{4r + p//32}[l], i.e. already the broadcast
            # layout needed to normalize OT.  One reciprocal evacuates it.
            dp = pmisc.tile([128, 512], F32, tag="misc")
            for h in range(HEADS):
                r, j = h // 4, h % 4
                nc.tensor.matmul(
                    dp[32 * j:32 * j + 32, r * 128:(r + 1) * 128],
                    lhsT=ones_s[:], rhs=es[:, h * 128:(h + 1) * 128],
                    start=(r == 0), stop=(r == 1), tile_position=(0, 32 * j),
                    skip_group_check=True,
                )
            dbs = pdbs.tile([128, 256], F32, tag="dbs")
            nc.vector.reciprocal_approx_fast(out=dbs[:], in_=dp[:, 0:256])

            # ---- attention output: OT_u = V_h.T-stationary @ E^T_h ----
            op_ = pmisc.tile([128, 512], F32, tag="misc")
            for h in range(HEADS):
                r, j = h // 4, h % 4
                nc.tensor.matmul(
                    op_[32 * j:32 * j + 32, r * 128:(r + 1) * 128],
                    lhsT=vs[:, 32 * h:32 * h + 32],
                    rhs=es[:, h * 128:(h + 1) * 128],
                    start=(r == 0), stop=(r == 1), tile_position=(0, 32 * j),
                    skip_group_check=True,
                )
            os_ = pos.tile([128, 256], F16, tag="os")
            nc.vector.scalar_tensor_tensor(
                out=os_[:], in0=op_[:, 0:256], scalar=1.0, in1=dbs[:],
                op0=ALU.mult, op1=ALU.mult,
            )

            # ---- output projection ZT = Wo OT ----
            zp = pmisc.tile([128, 512], F32, tag="misc")
            for coh in range(2):
                for kc in range(2):
                    nc.tensor.matmul(
                        zp[:, coh * 128:(coh + 1) * 128],
                        lhsT=wo_s[kc][:, coh * 128:(coh + 1) * 128],
                        rhs=os_[:, kc * 128:(kc + 1) * 128],
                        start=(coh == 0 and kc == 0),
                        stop=(coh == 1 and kc == 1),
                    )
            zs = pzs.tile([128, 256], F32, tag="zs")
            nc.scalar.activation(zs[:], zp[:, 0:256], AF.Copy)
            for coh in range(2):
                nc.sync.dma_start(
                    zt[w * 256 + coh * 128:w * 256 + (coh + 1) * 128, :],
                    zs[:, coh * 128:(coh + 1) * 128],
                )
    nc.compile()
    return nc


@functools.lru_cache(maxsize=2)
def _get_program(nw: int):
    return _build_program(nw)


def _im2win(x: np.ndarray) -> np.ndarray:
    """[B,T,H,W,C] -> [882,128,256] windows, flat order f = i_n*B + i_b."""
    s = x.strides
    xs = np.lib.stride_tricks.as_strided(
        x,
        shape=(B, T, NHW, PATCH, NHW, PATCH, C),
        strides=(s[0], s[1], STEP * s[2], s[2], STEP * s[3], s[3], s[4]),
    )
    w = xs.transpose(2, 4, 0, 1, 3, 5, 6)  # [iH,iW,b,t,p,q,c]
    return np.ascontiguousarray(w.reshape(NHW * NHW * B, L, C))


def _overlap_add(zwin: np.ndarray, bo: np.ndarray) -> np.ndarray:
    """[882,128,256] window outputs -> [B,T,H,W,C] with count-normalize + bo.

    Reproduces the reference's (n*b)->(b,n) flat-order reinterpretation.
    """
    th = np.arange(NHW) * STEP
    z = zwin.reshape(B, NHW, NHW, T, PATCH, PATCH, MID)  # [b,iH,iW,t,p,q,c]
    acc = np.zeros((B, T, H, W, MID), np.float32)
    count = np.zeros((H, W), np.float32)
    for p in range(PATCH):
        rid = (th + p)[:, None]
        for q in range(PATCH):
            cid = (th + q)[None, :]
            acc[:, :, rid, cid, :] += z[:, :, :, :, p, q, :].transpose(0, 3, 1, 2, 4)
            count[rid, cid] += 1.0
    out = acc / count[None, None, :, :, None] + bo[None, None, None, None, :]
    return out


LAST_RESULT = None


def kernel(x, ln_g, ln_b, Wq, Wk, Wv, Wo, bo):
    x = np.asarray(x, np.float32)
    ln_g = np.asarray(ln_g, np.float32)
    ln_b = np.asarray(ln_b, np.float32)
    assert np.allclose(ln_b, 0.0), "kernel folds ln_g into weights; ln_b must be 0"
    # Fold LN gamma into the input side of Wq/Wk/Wv.
    wq_t = np.ascontiguousarray((np.asarray(Wq, np.float32) * ln_g).T.astype(np.float16))
    wk_t = np.ascontiguousarray((np.asarray(Wk, np.float32) * ln_g).T.astype(np.float16))
    wv_t = np.ascontiguousarray((np.asarray(Wv, np.float32) * ln_g).T.astype(np.float16))
    wo_t = np.ascontiguousarray(np.asarray(Wo, np.float32).T.astype(np.float16))
    ones1 = np.ones((128, 32), np.float16)

    win = _im2win(x)                              # [882, 128, 256]
    pad = NCORES * NW - NWIN                      # 6
    winp = np.concatenate([win, np.zeros((pad, L, C), np.float32)], 0)
    shards = winp.reshape(NCORES, NW * L, C)

    nc = _get_program(NW)
    trace = bool(int(__import__("os").environ.get("KERNEL_TRACE", "0")))
    in_maps = []
    for i in range(NCORES):
        in_maps.append({
            "xw": np.ascontiguousarray(shards[i]),
            "wq": wq_t, "wk": wk_t, "wv": wv_t, "wo": wo_t,
            "ones1": ones1,
        })
    res = run_bass_kernel_spmd(nc, in_maps, core_ids=list(range(NCORES)),
                               trace=trace)
    global LAST_RESULT
    LAST_RESULT = res
    zts = [res.results[i]["zt"].reshape(NW, 2, 128, 128) for i in range(NCORES)]
    # zt rows: w*256 + c_out, cols l  ->  Z_w[l, c] = zt[w, :, :, l]
    zall = np.concatenate(zts, 0)                 # [888, 2, 128, 128]
    zwin = zall.reshape(NCORES * NW, MID, L).transpose(0, 2, 1)[:NWIN]
    return _overlap_add(np.ascontiguousarray(zwin), np.asarray(bo, np.float32))



# revision 7
# speedup vs baseline: 4.1882x; 4.1882x over previous
"""Trainium2 Bass kernel for windowed (sparse) attention — v3.

Module: LayerNorm -> overlapping 8x8 spatial windows (stride 6) over a
[2,2,128,128,256] image -> per-window 8-head attention over L=128 tokens
(t*8*8) -> output projection -> overlap-add with count normalization.

Strategy: 882 independent windows sharded over 8 cores (112 each, padded
to 896).  Host does im2win gather + overlap-add scatter; all model
compute runs on device.

v3 highlights (v1 = 1.82 ms, v2 = 598 us):
  - fp16 DRAM I/O; LN via bn_stats/bn_aggr
  - 1/sqrt(var+eps) via int-bit-trick + 2 Newton steps on the DVE,
    batched over 4-window groups -> the scalar engine runs ONLY
    Exp/Copy (one act table set, zero ACT_TABLE_LOAD thrash)
  - xn -> xnT via PE transpose (is_transpose matmul)
  - S matmuls K=128 with zero-padded head-major q ("qz", built by one
    gpsimd masked multiply); 4 heads per matmul (N=512, shared lhsT)
  - D (softmax denom) as 4 matmuls of N=256 via 3-dim rhs APs
  - softmax normalize via reciprocal_approx_fast + scalar_tensor_tensor
  - 11-deep software pipeline; PSUM fits exactly in 8 banks
"""

import functools
import math
from contextlib import ExitStack

import numpy as np

import concourse.bacc as bacc
import concourse.bass as bass
import concourse.mybir as mybir
import concourse.tile as tile
from concourse.bass import AP
from concourse.bass_utils import run_bass_kernel_spmd

# Problem constants (hardcoded per contract - kernel.py is self-contained).
B, T, H, W, C = 2, 2, 128, 128, 256
MID, HEADS = 256, 8
HD = MID // HEADS          # 32
PATCH, STEP = 8, 6         # window size / stride
NHW = 21                   # windows per axis: starts 0,6,...,120
NWIN = NHW * NHW * B       # 882 flat windows (n outer, b inner)
L = T * PATCH * PATCH      # 128 tokens per window
NCORES = 8
NW = 112                   # windows per core after padding to 896
EPS = 1e-6
MAGIC = 0x5F3759DF
F32, F16, I32 = mybir.dt.float32, mybir.dt.float16, mybir.dt.int32
AF = mybir.ActivationFunctionType
ALU = mybir.AluOpType


def _build_program(nw: int):
    nc = bacc.Bacc(
        "TRN2",
        target_bir_lowering=False,
        debug=False,
        enable_asserts=False,
        num_devices=NCORES,
    )
    xw = nc.dram_tensor("xw", [nw * 128, 256], F16, kind="ExternalInput").ap()
    wq = nc.dram_tensor("wq", [256, 256], F16, kind="ExternalInput").ap()
    wk = nc.dram_tensor("wk", [256, 256], F16, kind="ExternalInput").ap()
    wv = nc.dram_tensor("wv", [256, 256], F16, kind="ExternalInput").ap()
    wo = nc.dram_tensor("wo", [256, 256], F16, kind="ExternalInput").ap()
    ones1 = nc.dram_tensor("ones1", [128, 32], F16, kind="ExternalInput").ap()
    ident = nc.dram_tensor("ident", [128, 128], F16, kind="ExternalInput").ap()
    maskq = nc.dram_tensor("maskq", [128, 1024], F16, kind="ExternalInput").ap()
    zt = nc.dram_tensor("zt", [nw * 256, 128], F16, kind="ExternalOutput").ap()

    inv_sqrt_hd = 1.0 / math.sqrt(HD)
    GROUPS = nw // 4
    assert nw % 4 == 0

    with tile.TileContext(nc) as tc, ExitStack() as ctx:
        pw = ctx.enter_context(tc.tile_pool(name="wts", bufs=1))
        wq_s = [pw.tile([128, 256], F16, tag=f"wq{i}", name=f"wq{i}") for i in range(2)]
        wk_s = [pw.tile([128, 256], F16, tag=f"wk{i}", name=f"wk{i}") for i in range(2)]
        wv_s = [pw.tile([128, 256], F16, tag=f"wv{i}", name=f"wv{i}") for i in range(2)]
        wo_s = [pw.tile([128, 256], F16, tag=f"wo{i}", name=f"wo{i}") for i in range(2)]
        for i in range(2):
            nc.sync.dma_start(wq_s[i][:], wq[i * 128:(i + 1) * 128, :])
            nc.sync.dma_start(wk_s[i][:], wk[i * 128:(i + 1) * 128, :])
            nc.sync.dma_start(wv_s[i][:], wv[i * 128:(i + 1) * 128, :])
            nc.sync.dma_start(wo_s[i][:], wo[i * 128:(i + 1) * 128, :])
        ones_s = pw.tile([128, 32], F16, tag="ones1", name="ones1")
        nc.sync.dma_start(ones_s[:], ones1)
        id_s = pw.tile([128, 128], F16, tag="ident", name="ident")
        nc.sync.dma_start(id_s[:], ident)
        mq_s = pw.tile([128, 1024], F16, tag="maskq", name="maskq")
        nc.sync.dma_start(mq_s[:], maskq)

        # SBUF pools
        pxt = ctx.enter_context(tc.tile_pool(name="pxt", bufs=4))
        pxn = ctx.enter_context(tc.tile_pool(name="pxn", bufs=3))
        pxnt = ctx.enter_context(tc.tile_pool(name="pxnt", bufs=3))
        pqks = ctx.enter_context(tc.tile_pool(name="pqks", bufs=3))
        pqz = ctx.enter_context(tc.tile_pool(name="pqz", bufs=3))
        pvs = ctx.enter_context(tc.tile_pool(name="pvs", bufs=4))
        pes = ctx.enter_context(tc.tile_pool(name="pes", bufs=3))
        pos = ctx.enter_context(tc.tile_pool(name="pos", bufs=3))
        pzs = ctx.enter_context(tc.tile_pool(name="pzs", bufs=3))
        pst = ctx.enter_context(tc.tile_pool(name="pst", bufs=3))
        pch = ctx.enter_context(tc.tile_pool(name="pch", bufs=2))
        # PSUM pools: 1 + 1 + 1 + 2 + 3 = 8 banks
        ptp = ctx.enter_context(tc.tile_pool(name="ptp", bufs=1, space="PSUM"))
        pqk = ctx.enter_context(tc.tile_pool(name="pqk", bufs=1, space="PSUM"))
        pv = ctx.enter_context(tc.tile_pool(name="pv", bufs=1, space="PSUM"))
        psp = ctx.enter_context(tc.tile_pool(name="psp", bufs=1, space="PSUM"))
        pmisc = ctx.enter_context(tc.tile_pool(name="pmisc", bufs=3, space="PSUM"))

        # Pipeline state
        xt_g = {}
        bag_g, rs4_g = {}, {}
        xn_t, xnt_t, qks_t, qz_t, vs_t, es_t, os_t = {}, {}, {}, {}, {}, {}, {}

        def load_group(g):
            t = pxt.tile([128, 1024], F16, tag="xt", name="xt")
            src = AP(xw.tensor, g * 4 * 128 * 256,
                     [[256, 128], [128 * 256, 4], [1, 256]])
            nc.sync.dma_start(t[:], src)
            xt_g[g] = t
            xt_g.pop(g - 3, None)

        load_group(0)

        for i in range(nw + 11):
            if (i + 2) % 4 == 0:
                g = (i + 2) // 4
                if g < GROUPS:
                    load_group(g)

            # ---- s0 (w=i): LN stats into group tile ----
            w = i
            if w < nw:
                g = w // 4
                if w % 4 == 0:
                    bag_g[g] = pst.tile([128, 8], F32, tag="bagg", name="bagg")
                xt = xt_g[g]
                xs = xt[:, (w % 4) * 256:(w % 4 + 1) * 256]
                bst = pst.tile([128, 6], F32, tag="bst", name="bst")
                nc.vector.bn_stats(bst[:], xs)
                nc.vector.bn_aggr(bag_g[g][:, 2 * (w % 4):2 * (w % 4) + 2], bst[:])

            # ---- group chain: rsqrt of 4 vars at once (iter 4g+4) ----
            if i % 4 == 0 and i // 4 - 1 >= 0 and i // 4 - 1 < GROUPS:
                g = i // 4 - 1
                bag = bag_g[g]
                var4 = AP(bag[:].tensor, bag[:].offset + 1, [[8, 128], [2, 4]])
                ve = pch.tile([128, 4], F32, tag="ve", name="ve")
                nc.vector.tensor_scalar(out=ve[:], in0=var4, scalar1=EPS,
                                        scalar2=None, op0=ALU.add)
                ti = pch.tile([128, 4], I32, tag="ti", name="ti")
                nc.vector.tensor_scalar(out=ti[:], in0=ve[:].bitcast(I32),
                                        scalar1=1, scalar2=None,
                                        op0=ALU.arith_shift_right)
                y0i = pch.tile([128, 4], I32, tag="y0i", name="y0i")
                nc.vector.tensor_scalar(out=y0i[:], in0=ti[:], scalar1=-1,
                                        scalar2=MAGIC, op0=ALU.mult, op1=ALU.add)
                y0 = y0i[:].bitcast(F32)
                u = pch.tile([128, 4], F32, tag="u", name="u")
                nc.vector.tensor_tensor(out=u[:], in0=y0, in1=y0, op=ALU.mult)
                a = pch.tile([128, 4], F32, tag="a", name="a")
                nc.vector.scalar_tensor_tensor(out=a[:], in0=u[:], scalar=-0.5,
                                               op0=ALU.mult, op1=ALU.mult,
                                               in1=ve[:])
                y1 = pch.tile([128, 4], F32, tag="y1", name="y1")
                nc.vector.scalar_tensor_tensor(out=y1[:], in0=a[:], scalar=1.5,
                                               op0=ALU.add, op1=ALU.mult, in1=y0)
                u2 = pch.tile([128, 4], F32, tag="u2", name="u2")
                nc.vector.tensor_tensor(out=u2[:], in0=y1[:], in1=y1[:],
                                        op=ALU.mult)
                a2 = pch.tile([128, 4], F32, tag="a2", name="a2")
                nc.vector.scalar_tensor_tensor(out=a2[:], in0=u2[:], scalar=-0.5,
                                               op0=ALU.mult, op1=ALU.mult,
                                               in1=ve[:])
                rs4 = pch.tile([128, 4], F32, tag="rs4", name="rs4")
                nc.vector.scalar_tensor_tensor(out=rs4[:], in0=a2[:], scalar=1.5,
                                               op0=ALU.add, op1=ALU.mult,
                                               in1=y1[:])
                rs4_g[g] = rs4

            # ---- s_apply (w=i-5): LN apply -> xn f16 ----
            w = i - 5
            if 0 <= w < nw:
                g = w // 4
                xt = xt_g[g]
                xs = xt[:, (w % 4) * 256:(w % 4 + 1) * 256]
                xn = pxn.tile([128, 256], F16, tag="xn", name="xn")
                nc.vector.tensor_scalar(
                    out=xn[:], in0=xs,
                    scalar1=bag_g[g][:, 2 * (w % 4):2 * (w % 4) + 1],
                    scalar2=rs4_g[g][:, w % 4:w % 4 + 1],
                    op0=ALU.subtract, op1=ALU.mult,
                )
                xn_t[w] = xn
                if w % 4 == 3:
                    bag_g.pop(g, None)
                    rs4_g.pop(g, None)

            # ---- s_T (w=i-6): PE transpose + evac ----
            w = i - 6
            if 0 <= w < nw:
                tp = ptp.tile([128, 1024], F16, tag="tp", name="tp")
                for kc in range(2):
                    nc.tensor.transpose(
                        tp[:, kc * 128:(kc + 1) * 128],
                        xn_t[w][:, kc * 128:(kc + 1) * 128], id_s[:])
                xnt = pxnt.tile([128, 256], F16, tag="xnt", name="xnt")
                nc.vector.tensor_copy(xnt[:], tp[:, 0:256])
                xnt_t[w] = xnt
                del xn_t[w]

            # ---- s_qkv (w=i-7): projections + evacs + qz ----
            w = i - 7
            if 0 <= w < nw:
                xnt = xnt_t[w]
                qkp = pqk.tile([128, 512], F32, tag="qk", name="qk")
                first = True
                for ws, base in ((wq_s, 0), (wk_s, 256)):
                    for mh in range(2):
                        for kc in range(2):
                            nc.tensor.matmul(
                                qkp[:, base + mh * 128:base + (mh + 1) * 128],
                                lhsT=ws[kc][:, mh * 128:(mh + 1) * 128],
                                rhs=xnt[:, kc * 128:(kc + 1) * 128],
                                start=first,
                                stop=(base == 256 and mh == 1 and kc == 1),
                            )
                            first = False
                vp = pv.tile([128, 512], F32, tag="v", name="v")
                for kc in range(2):
                    nc.tensor.matmul(
                        vp[:, 0:256], lhsT=xnt[:, kc * 128:(kc + 1) * 128],
                        rhs=wv_s[kc][:], start=(kc == 0), stop=(kc == 1),
                    )
                qks = pqks.tile([128, 512], F16, tag="qks", name="qks")
                nc.scalar.copy(qks[:], qkp[:])
                vs = pvs.tile([128, 256], F16, tag="vs", name="vs")
                nc.vector.tensor_copy(vs[:], vp[:, 0:256])
                qz = pqz.tile([128, 1024], F16, tag="qz", name="qz")
                qrep = AP(qks[:].tensor, qks[:].offset,
                          [[512, 128], [128, 2], [0, 4], [1, 128]])
                nc.gpsimd.tensor_tensor(out=qz[:], in0=qrep, in1=mq_s[:],
                                        op=ALU.mult)
                qks_t[w], qz_t[w], vs_t[w] = qks, qz, vs
                del xnt_t[w]

            # ---- s_S (w=i-8): S matmuls (4 heads per instr) + exp ----
            w = i - 8
            if 0 <= w < nw:
                qks, qz = qks_t[w], qz_t[w]
                sp = psp.tile([128, 1024], F32, tag="sp", name="sp")
                for mh in range(2):
                    nc.tensor.matmul(
                        sp[:, mh * 512:(mh + 1) * 512],
                        lhsT=qks[:, 256 + mh * 128:256 + (mh + 1) * 128],
                        rhs=qz[:, mh * 512:(mh + 1) * 512],
                        start=True, stop=True,
                    )
                es = pes.tile([128, 1024], F16, tag="es", name="es")
                nc.scalar.activation(es[:], sp[:], AF.Exp, scale=inv_sqrt_hd)
                es_t[w] = es
                del qks_t[w], qz_t[w]

            # ---- s_DOT (w=i-9): D (4x N=256) + OT (8x) + normalize ----
            w = i - 9
            if 0 <= w < nw:
                es, vs = es_t[w], vs_t[w]
                dp = pmisc.tile([128, 512], F32, tag="misc", name="misc")
                for j in range(4):
                    rhs = AP(es[:].tensor, es[:].offset + j * 128,
                             [[1024, 128], [512, 2], [1, 128]])
                    nc.tensor.matmul(
                        dp[32 * j:32 * j + 32, 0:256],
                        lhsT=ones_s[:], rhs=rhs,
                        start=True, stop=True, tile_position=(0, 32 * j),
                        skip_group_check=True,
                    )
                op_ = pmisc.tile([128, 512], F32, tag="misc", name="misc")
                for h in range(HEADS):
                    r, j = h // 4, h % 4
                    nc.tensor.matmul(
                        op_[32 * j:32 * j + 32, r * 128:(r + 1) * 128],
                        lhsT=vs[:, 32 * h:32 * h + 32],
                        rhs=es[:, h * 128:(h + 1) * 128],
                        start=(r == 0), stop=(r == 1), tile_position=(0, 32 * j),
                        skip_group_check=True,
                    )
                dbs = pst.tile([128, 256], F32, tag="dbs", name="dbs")
                nc.vector.reciprocal_approx_fast(out=dbs[:], in_=dp[:, 0:256])
                os_ = pos.tile([128, 256], F16, tag="os", name="os")
                nc.vector.scalar_tensor_tensor(
                    out=os_[:], in0=op_[:, 0:256], scalar=1.0, in1=dbs[:],
                    op0=ALU.mult, op1=ALU.mult,
                )
                os_t[w] = os_
                del es_t[w], vs_t[w]

            # ---- s_Z (w=i-10): out projection + store ----
            w = i - 10
            if 0 <= w < nw:
                os_ = os_t[w]
                zp = pmisc.tile([128, 512], F32, tag="misc", name="misc")
                for coh in range(2):
                    for kc in range(2):
                        nc.tensor.matmul(
                            zp[:, coh * 128:(coh + 1) * 128],
                            lhsT=wo_s[kc][:, coh * 128:(coh + 1) * 128],
                            rhs=os_[:, kc * 128:(kc + 1) * 128],
                            start=(coh == 0 and kc == 0),
                            stop=(coh == 1 and kc == 1),
                        )
                zs = pzs.tile([128, 256], F16, tag="zs", name="zs")
                nc.scalar.copy(zs[:], zp[:, 0:256])
                dst = AP(zt.tensor, w * 256 * 128,
                         [[128, 128], [128 * 128, 2], [1, 128]])
                nc.sync.dma_start(dst, zs[:])
                del os_t[w]
    nc.compile()
    return nc


@functools.lru_cache(maxsize=2)
def _get_program(nw: int):
    return _build_program(nw)


def _im2win(x: np.ndarray) -> np.ndarray:
    """[B,T,H,W,C] -> [882,128,256] windows, flat order f = i_n*B + i_b."""
    s = x.strides
    xs = np.lib.stride_tricks.as_strided(
        x,
        shape=(B, T, NHW, PATCH, NHW, PATCH, C),
        strides=(s[0], s[1], STEP * s[2], s[2], STEP * s[3], s[3], s[4]),
    )
    w = xs.transpose(2, 4, 0, 1, 3, 5, 6)  # [iH,iW,b,t,p,q,c]
    return np.ascontiguousarray(w.reshape(NHW * NHW * B, L, C))


def _overlap_add(zwin: np.ndarray, bo: np.ndarray) -> np.ndarray:
    """[882,128,256] window outputs -> [B,T,H,W,C] with count-normalize + bo."""
    th = np.arange(NHW) * STEP
    z = zwin.reshape(B, NHW, NHW, T, PATCH, PATCH, MID)  # [b,iH,iW,t,p,q,c]
    acc = np.zeros((B, T, H, W, MID), np.float32)
    count = np.zeros((H, W), np.float32)
    for p in range(PATCH):
        rid = (th + p)[:, None]
        for q in range(PATCH):
            cid = (th + q)[None, :]
            acc[:, :, rid, cid, :] += z[:, :, :, :, p, q, :].transpose(0, 3, 1, 2, 4)
            count[rid, cid] += 1.0
    out = acc / count[None, None, :, :, None] + bo[None, None, None, None, :]
    return out


LAST_RESULT = None


def kernel(x, ln_g, ln_b, Wq, Wk, Wv, Wo, bo):
    x = np.asarray(x, np.float32)
    ln_g = np.asarray(ln_g, np.float32)
    ln_b = np.asarray(ln_b, np.float32)
    assert np.allclose(ln_b, 0.0), "kernel folds ln_g into weights; ln_b must be 0"
    # Fold LN gamma into the input side of Wq/Wk/Wv.
    wq_t = np.ascontiguousarray((np.asarray(Wq, np.float32) * ln_g).T.astype(np.float16))
    wk_t = np.ascontiguousarray((np.asarray(Wk, np.float32) * ln_g).T.astype(np.float16))
    wv_t = np.ascontiguousarray((np.asarray(Wv, np.float32) * ln_g).T.astype(np.float16))
    wo_t = np.ascontiguousarray(np.asarray(Wo, np.float32).T.astype(np.float16))
    ones1 = np.ones((128, 32), np.float16)
    ident = np.eye(128, dtype=np.float16)
    maskq = np.zeros((128, 1024), np.float16)
    for h in range(HEADS):
        j = h % 4
        maskq[32 * j:32 * j + 32, h * 128:(h + 1) * 128] = 1.0

    win = _im2win(x)                              # [882, 128, 256]
    pad = NCORES * NW - NWIN                      # 14
    winp = np.concatenate([win, np.zeros((pad, L, C), np.float32)], 0)
    shards = winp.reshape(NCORES, NW * L, C)

    nc = _get_program(NW)
    trace = bool(int(__import__("os").environ.get("KERNEL_TRACE", "0")))
    in_maps = []
    for i in range(NCORES):
        in_maps.append({
            "xw": np.ascontiguousarray(shards[i]).astype(np.float16),
            "wq": wq_t, "wk": wk_t, "wv": wv_t, "wo": wo_t,
            "ones1": ones1, "ident": ident, "maskq": maskq,
        })
    res = run_bass_kernel_spmd(nc, in_maps, core_ids=list(range(NCORES)),
                               trace=trace)
    global LAST_RESULT
    LAST_RESULT = res
    zts = [np.asarray(res.results[i]["zt"], np.float32).reshape(NW, 2, 128, 128)
           for i in range(NCORES)]
    # zt rows: w*256 + c_out, cols l  ->  Z_w[l, c] = zt[w, :, :, l]
    zall = np.concatenate(zts, 0)                 # [896, 2, 128, 128]
    zwin = zall.reshape(NCORES * NW, MID, L).transpose(0, 2, 1)[:NWIN]
    return _overlap_add(np.ascontiguousarray(zwin), np.asarray(bo, np.float32))


# revision 10
# speedup vs baseline: 4.2024x; 1.0034x over previous
"""Trainium2 Bass kernel for windowed (sparse) attention — v3.

Module: LayerNorm -> overlapping 8x8 spatial windows (stride 6) over a
[2,2,128,128,256] image -> per-window 8-head attention over L=128 tokens
(t*8*8) -> output projection -> overlap-add with count normalization.

Strategy: 882 independent windows sharded over 8 cores (112 each, padded
to 896).  Host does im2win gather + overlap-add scatter; all model
compute runs on device.

v3 highlights (v1 = 1.82 ms, v2 = 598 us):
  - fp16 DRAM I/O; LN via bn_stats/bn_aggr
  - 1/sqrt(var+eps) via int-bit-trick + 2 Newton steps on the DVE,
    batched over 4-window groups -> the scalar engine runs ONLY
    Exp/Copy (one act table set, zero ACT_TABLE_LOAD thrash)
  - xn -> xnT via PE transpose (is_transpose matmul)
  - S matmuls K=128 with zero-padded head-major q ("qz", built by one
    gpsimd masked multiply); 4 heads per matmul (N=512, shared lhsT)
  - D (softmax denom) as 4 matmuls of N=256 via 3-dim rhs APs
  - softmax normalize via reciprocal_approx_fast + scalar_tensor_tensor
  - 11-deep software pipeline; PSUM fits exactly in 8 banks
"""

import functools
import math
from contextlib import ExitStack

import numpy as np

import concourse.bacc as bacc
import concourse.bass as bass
import concourse.mybir as mybir
import concourse.tile as tile
from concourse.bass import AP
from concourse.bass_utils import run_bass_kernel_spmd

# Force every ACT function this kernel uses (Exp, Ln, Copy) into the single
# combined table set `natural_log_exp_and_others` so the scalar engine never
# swaps activation tables (each swap costs ~2.7us).  Indices into
# act_info.json must be preserved, so we strip these functions from every
# other set rather than reordering.
_ORIG_GAT = bacc.get_activation_tables

def _patched_gat(arch):
    tabs = _ORIG_GAT(arch)
    strip = {mybir.ActivationFunctionType.from_pwp(n)
             for n in ("exp", "ln", "copy", "identity")}
    return {n: (fns if n == "natural_log_exp_and_others" else fns - strip)
            for n, fns in tabs.items()}

bacc.get_activation_tables = _patched_gat

# Problem constants (hardcoded per contract - kernel.py is self-contained).
B, T, H, W, C = 2, 2, 128, 128, 256
MID, HEADS = 256, 8
HD = MID // HEADS          # 32
PATCH, STEP = 8, 6         # window size / stride
NHW = 21                   # windows per axis: starts 0,6,...,120
NWIN = NHW * NHW * B       # 882 flat windows (n outer, b inner)
L = T * PATCH * PATCH      # 128 tokens per window
NCORES = 8
NW = 112                   # windows per core after padding to 896
EPS = 1e-6
MAGIC = 0x5F3759DF
F32, F16, I32 = mybir.dt.float32, mybir.dt.float16, mybir.dt.int32
AF = mybir.ActivationFunctionType
ALU = mybir.AluOpType


def _build_program(nw: int):
    nc = bacc.Bacc(
        "TRN2",
        target_bir_lowering=False,
        debug=False,
        enable_asserts=False,
        num_devices=NCORES,
    )
    xw = nc.dram_tensor("xw", [nw * 128, 256], F16, kind="ExternalInput").ap()
    wq = nc.dram_tensor("wq", [256, 256], F16, kind="ExternalInput").ap()
    wk = nc.dram_tensor("wk", [256, 256], F16, kind="ExternalInput").ap()
    wv = nc.dram_tensor("wv", [256, 256], F16, kind="ExternalInput").ap()
    wo = nc.dram_tensor("wo", [256, 256], F16, kind="ExternalInput").ap()
    ones1 = nc.dram_tensor("ones1", [128, 32], F16, kind="ExternalInput").ap()
    ident = nc.dram_tensor("ident", [128, 128], F16, kind="ExternalInput").ap()
    maskq = nc.dram_tensor("maskq", [128, 1024], F16, kind="ExternalInput").ap()
    zt = nc.dram_tensor("zt", [nw * 256, 128], F16, kind="ExternalOutput").ap()

    inv_sqrt_hd = 1.0 / math.sqrt(HD)
    GROUPS = nw // 4
    assert nw % 4 == 0

    with tile.TileContext(nc) as tc, ExitStack() as ctx:
        pw = ctx.enter_context(tc.tile_pool(name="wts", bufs=1))
        wq_s = [pw.tile([128, 256], F16, tag=f"wq{i}", name=f"wq{i}") for i in range(2)]
        wk_s = [pw.tile([128, 256], F16, tag=f"wk{i}", name=f"wk{i}") for i in range(2)]
        wv_s = [pw.tile([128, 256], F16, tag=f"wv{i}", name=f"wv{i}") for i in range(2)]
        wo_s = [pw.tile([128, 256], F16, tag=f"wo{i}", name=f"wo{i}") for i in range(2)]
        for i in range(2):
            nc.sync.dma_start(wq_s[i][:], wq[i * 128:(i + 1) * 128, :])
            nc.sync.dma_start(wk_s[i][:], wk[i * 128:(i + 1) * 128, :])
            nc.sync.dma_start(wv_s[i][:], wv[i * 128:(i + 1) * 128, :])
            nc.sync.dma_start(wo_s[i][:], wo[i * 128:(i + 1) * 128, :])
        ones_s = pw.tile([128, 32], F16, tag="ones1", name="ones1")
        nc.sync.dma_start(ones_s[:], ones1)
        id_s = pw.tile([128, 128], F16, tag="ident", name="ident")
        nc.sync.dma_start(id_s[:], ident)
        mq_s = pw.tile([128, 1024], F16, tag="maskq", name="maskq")
        nc.sync.dma_start(mq_s[:], maskq)
        eps_s = pw.tile([128, 1], F32, tag="eps", name="eps")
        nc.vector.memset(eps_s[:], EPS)

        # SBUF pools
        pxt = ctx.enter_context(tc.tile_pool(name="pxt", bufs=4))
        pxn = ctx.enter_context(tc.tile_pool(name="pxn", bufs=3))
        pxnt = ctx.enter_context(tc.tile_pool(name="pxnt", bufs=3))
        pqks = ctx.enter_context(tc.tile_pool(name="pqks", bufs=3))
        pqz = ctx.enter_context(tc.tile_pool(name="pqz", bufs=3))
        pvs = ctx.enter_context(tc.tile_pool(name="pvs", bufs=4))
        pes = ctx.enter_context(tc.tile_pool(name="pes", bufs=3))
        pos = ctx.enter_context(tc.tile_pool(name="pos", bufs=3))
        pzs = ctx.enter_context(tc.tile_pool(name="pzs", bufs=3))
        pst = ctx.enter_context(tc.tile_pool(name="pst", bufs=3))
        pch = ctx.enter_context(tc.tile_pool(name="pch", bufs=2))
        # PSUM pools: 1 + 1 + 1 + 2 + 3 = 8 banks
        ptp = ctx.enter_context(tc.tile_pool(name="ptp", bufs=1, space="PSUM"))
        pqk = ctx.enter_context(tc.tile_pool(name="pqk", bufs=1, space="PSUM"))
        pv = ctx.enter_context(tc.tile_pool(name="pv", bufs=1, space="PSUM"))
        psp = ctx.enter_context(tc.tile_pool(name="psp", bufs=1, space="PSUM"))
        pmisc = ctx.enter_context(tc.tile_pool(name="pmisc", bufs=3, space="PSUM"))

        # Pipeline state
        xt_g = {}
        bag_g, rs4_g = {}, {}
        xn_t, xnt_t, qks_t, qz_t, vs_t, es_t, os_t = {}, {}, {}, {}, {}, {}, {}

        def load_group(g):
            t = pxt.tile([128, 1024], F16, tag="xt", name="xt")
            src = AP(xw.tensor, g * 4 * 128 * 256,
                     [[256, 128], [128 * 256, 4], [1, 256]])
            nc.sync.dma_start(t[:], src)
            xt_g[g] = t
            xt_g.pop(g - 3, None)

        load_group(0)

        for i in range(nw + 11):
            if (i + 2) % 4 == 0:
                g = (i + 2) // 4
                if g < GROUPS:
                    load_group(g)

            # ---- s0 (w=i): LN stats into group tile ----
            w = i
            if w < nw:
                g = w // 4
                if w % 4 == 0:
                    bag_g[g] = pst.tile([128, 8], F32, tag="bagg", name="bagg")
                xt = xt_g[g]
                xs = xt[:, (w % 4) * 256:(w % 4 + 1) * 256]
                bst = pst.tile([128, 6], F32, tag="bst", name="bst")
                nc.vector.bn_stats(bst[:], xs)
                nc.vector.bn_aggr(bag_g[g][:, 2 * (w % 4):2 * (w % 4) + 2], bst[:])

            # ---- group rsqrt: rs4 = Exp(-0.5*Ln(var4+eps)), 4 windows/op ----
            if i % 4 == 0 and i // 4 - 1 >= 0 and i // 4 - 1 < GROUPS:
                g = i // 4 - 1
                bag = bag_g[g]
                var4 = AP(bag[:].tensor, bag[:].offset + 1, [[8, 128], [2, 4]])
                lnv = pch.tile([128, 4], F32, tag="lnv", name="lnv")
                nc.scalar.activation(lnv[:], var4, AF.Ln, bias=eps_s[:])
                rs4 = pch.tile([128, 4], F32, tag="rs4", name="rs4")
                nc.scalar.activation(rs4[:], lnv[:], AF.Exp, scale=-0.5)
                rs4_g[g] = rs4

            # ---- s_apply (w=i-5): LN apply -> xn f16 ----
            w = i - 5
            if 0 <= w < nw:
                g = w // 4
                xt = xt_g[g]
                xs = xt[:, (w % 4) * 256:(w % 4 + 1) * 256]
                xn = pxn.tile([128, 256], F16, tag="xn", name="xn")
                nc.vector.tensor_scalar(
                    out=xn[:], in0=xs,
                    scalar1=bag_g[g][:, 2 * (w % 4):2 * (w % 4) + 1],
                    scalar2=rs4_g[g][:, w % 4:w % 4 + 1],
                    op0=ALU.subtract, op1=ALU.mult,
                )
                xn_t[w] = xn
                if w % 4 == 3:
                    bag_g.pop(g, None)
                    rs4_g.pop(g, None)

            # ---- s_T (w=i-6): PE transpose + evac ----
            w = i - 6
            if 0 <= w < nw:
                tp = ptp.tile([128, 1024], F16, tag="tp", name="tp")
                for kc in range(2):
                    nc.tensor.transpose(
                        tp[:, kc * 128:(kc + 1) * 128],
                        xn_t[w][:, kc * 128:(kc + 1) * 128], id_s[:])
                xnt = pxnt.tile([128, 256], F16, tag="xnt", name="xnt")
                nc.vector.tensor_copy(xnt[:], tp[:, 0:256])
                xnt_t[w] = xnt
                del xn_t[w]

            # ---- s_qkv (w=i-7): projections + evacs + qz ----
            w = i - 7
            if 0 <= w < nw:
                xnt = xnt_t[w]
                qkp = pqk.tile([128, 512], F32, tag="qk", name="qk")
                first = True
                for ws, base in ((wq_s, 0), (wk_s, 256)):
                    for mh in range(2):
                        for kc in range(2):
                            nc.tensor.matmul(
                                qkp[:, base + mh * 128:base + (mh + 1) * 128],
                                lhsT=ws[kc][:, mh * 128:(mh + 1) * 128],
                                rhs=xnt[:, kc * 128:(kc + 1) * 128],
                                start=first,
                                stop=(base == 256 and mh == 1 and kc == 1),
                            )
                            first = False
                vp = pv.tile([128, 512], F32, tag="v", name="v")
                for kc in range(2):
                    nc.tensor.matmul(
                        vp[:, 0:256], lhsT=xnt[:, kc * 128:(kc + 1) * 128],
                        rhs=wv_s[kc][:], start=(kc == 0), stop=(kc == 1),
                    )
                qks = pqks.tile([128, 512], F16, tag="qks", name="qks")
                nc.scalar.copy(qks[:], qkp[:])
                vs = pvs.tile([128, 256], F16, tag="vs", name="vs")
                nc.vector.tensor_copy(vs[:], vp[:, 0:256])
                qz = pqz.tile([128, 1024], F16, tag="qz", name="qz")
                qrep = AP(qks[:].tensor, qks[:].offset,
                          [[512, 128], [128, 2], [0, 4], [1, 128]])
                nc.gpsimd.tensor_tensor(out=qz[:], in0=qrep, in1=mq_s[:],
                                        op=ALU.mult)
                qks_t[w], qz_t[w], vs_t[w] = qks, qz, vs
                del xnt_t[w]

            # ---- s_S (w=i-8): S matmuls (4 heads per instr) + exp ----
            w = i - 8
            if 0 <= w < nw:
                qks, qz = qks_t[w], qz_t[w]
                sp = psp.tile([128, 1024], F32, tag="sp", name="sp")
                for mh in range(2):
                    nc.tensor.matmul(
                        sp[:, mh * 512:(mh + 1) * 512],
                        lhsT=qks[:, 256 + mh * 128:256 + (mh + 1) * 128],
                        rhs=qz[:, mh * 512:(mh + 1) * 512],
                        start=True, stop=True,
                    )
                es = pes.tile([128, 1024], F16, tag="es", name="es")
                nc.scalar.activation(es[:], sp[:], AF.Exp, scale=inv_sqrt_hd)
                es_t[w] = es
                del qks_t[w], qz_t[w]

            # ---- s_DOT (w=i-9): D (4x N=256) + OT (8x) + normalize ----
            w = i - 9
            if 0 <= w < nw:
                es, vs = es_t[w], vs_t[w]
                dp = pmisc.tile([128, 512], F32, tag="misc", name="misc")
                for j in range(4):
                    rhs = AP(es[:].tensor, es[:].offset + j * 128,
                             [[1024, 128], [512, 2], [1, 128]])
                    nc.tensor.matmul(
                        dp[32 * j:32 * j + 32, 0:256],
                        lhsT=ones_s[:], rhs=rhs,
                        start=True, stop=True, tile_position=(0, 32 * j),
                        skip_group_check=True,
                    )
                op_ = pmisc.tile([128, 512], F32, tag="misc", name="misc")
                for h in range(HEADS):
                    r, j = h // 4, h % 4
                    nc.tensor.matmul(
                        op_[32 * j:32 * j + 32, r * 128:(r + 1) * 128],
                        lhsT=vs[:, 32 * h:32 * h + 32],
                        rhs=es[:, h * 128:(h + 1) * 128],
                        start=(r == 0), stop=(r == 1), tile_position=(0, 32 * j),
                        skip_group_check=True,
                    )
                dbs = pst.tile([128, 256], F32, tag="dbs", name="dbs")
                nc.vector.reciprocal_approx_fast(out=dbs[:], in_=dp[:, 0:256])
                os_ = pos.tile([128, 256], F16, tag="os", name="os")
                nc.vector.scalar_tensor_tensor(
                    out=os_[:], in0=op_[:, 0:256], scalar=1.0, in1=dbs[:],
                    op0=ALU.mult, op1=ALU.mult,
                )
                os_t[w] = os_
                del es_t[w], vs_t[w]

            # ---- s_Z (w=i-10): out projection + store ----
            w = i - 10
            if 0 <= w < nw:
                os_ = os_t[w]
                zp = pmisc.tile([128, 512], F32, tag="misc", name="misc")
                for coh in range(2):
                    for kc in range(2):
                        nc.tensor.matmul(
                            zp[:, coh * 128:(coh + 1) * 128],
                            lhsT=wo_s[kc][:, coh * 128:(coh + 1) * 128],
                            rhs=os_[:, kc * 128:(kc + 1) * 128],
                            start=(coh == 0 and kc == 0),
                            stop=(coh == 1 and kc == 1),
                        )
                zs = pzs.tile([128, 256], F16, tag="zs", name="zs")
                nc.scalar.copy(zs[:], zp[:, 0:256])
                dst = AP(zt.tensor, w * 256 * 128,
                         [[128, 128], [128 * 128, 2], [1, 128]])
                nc.sync.dma_start(dst, zs[:])
                del os_t[w]
    nc.compile()
    return nc


@functools.lru_cache(maxsize=2)
def _get_program(nw: int):
    return _build_program(nw)


def _im2win(x: np.ndarray) -> np.ndarray:
    """[B,T,H,W,C] -> [882,128,256] windows, flat order f = i_n*B + i_b."""
    s = x.strides
    xs = np.lib.stride_tricks.as_strided(
        x,
        shape=(B, T, NHW, PATCH, NHW, PATCH, C),
        strides=(s[0], s[1], STEP * s[2], s[2], STEP * s[3], s[3], s[4]),
    )
    w = xs.transpose(2, 4, 0, 1, 3, 5, 6)  # [iH,iW,b,t,p,q,c]
    return np.ascontiguousarray(w.reshape(NHW * NHW * B, L, C))


def _overlap_add(zwin: np.ndarray, bo: np.ndarray) -> np.ndarray:
    """[882,128,256] window outputs -> [B,T,H,W,C] with count-normalize + bo."""
    th = np.arange(NHW) * STEP
    z = zwin.reshape(B, NHW, NHW, T, PATCH, PATCH, MID)  # [b,iH,iW,t,p,q,c]
    acc = np.zeros((B, T, H, W, MID), np.float32)
    count = np.zeros((H, W), np.float32)
    for p in range(PATCH):
        rid = (th + p)[:, None]
        for q in range(PATCH):
            cid = (th + q)[None, :]
            acc[:, :, rid, cid, :] += z[:, :, :, :, p, q, :].transpose(0, 3, 1, 2, 4)
            count[rid, cid] += 1.0
    out = acc / count[None, None, :, :, None] + bo[None, None, None, None, :]
    return out


LAST_RESULT = None


def kernel(x, ln_g, ln_b, Wq, Wk, Wv, Wo, bo):
    x = np.asarray(x, np.float32)
    ln_g = np.asarray(ln_g, np.float32)
    ln_b = np.asarray(ln_b, np.float32)
    assert np.allclose(ln_b, 0.0), "kernel folds ln_g into weights; ln_b must be 0"
    # Fold LN gamma into the input side of Wq/Wk/Wv.
    wq_t = np.ascontiguousarray((np.asarray(Wq, np.float32) * ln_g).T.astype(np.float16))
    wk_t = np.ascontiguousarray((np.asarray(Wk, np.float32) * ln_g).T.astype(np.float16))
    wv_t = np.ascontiguousarray((np.asarray(Wv, np.float32) * ln_g).T.astype(np.float16))
    wo_t = np.ascontiguousarray(np.asarray(Wo, np.float32).T.astype(np.float16))
    ones1 = np.ones((128, 32), np.float16)
    ident = np.eye(128, dtype=np.float16)
    maskq = np.zeros((128, 1024), np.float16)
    for h in range(HEADS):
        j = h % 4
        maskq[32 * j:32 * j + 32, h * 128:(h + 1) * 128] = 1.0

    win = _im2win(x)                              # [882, 128, 256]
    pad = NCORES * NW - NWIN                      # 14
    winp = np.concatenate([win, np.zeros((pad, L, C), np.float32)], 0)
    shards = winp.reshape(NCORES, NW * L, C)

    nc = _get_program(NW)
    trace = bool(int(__import__("os").environ.get("KERNEL_TRACE", "0")))
    in_maps = []
    for i in range(NCORES):
        in_maps.append({
            "xw": np.ascontiguousarray(shards[i]).astype(np.float16),
            "wq": wq_t, "wk": wk_t, "wv": wv_t, "wo": wo_t,
            "ones1": ones1, "ident": ident, "maskq": maskq,
        })
    res = run_bass_kernel_spmd(nc, in_maps, core_ids=list(range(NCORES)),
                               trace=trace)
    global LAST_RESULT
    LAST_RESULT = res
    zts = [np.asarray(res.results[i]["zt"], np.float32).reshape(NW, 2, 128, 128)
           for i in range(NCORES)]
    # zt rows: w*256 + c_out, cols l  ->  Z_w[l, c] = zt[w, :, :, l]
    zall = np.concatenate(zts, 0)                 # [896, 2, 128, 128]
    zwin = zall.reshape(NCORES * NW, MID, L).transpose(0, 2, 1)[:NWIN]
    return _overlap_add(np.ascontiguousarray(zwin), np.asarray(bo, np.float32))


# revision 12
# speedup vs baseline: 5.2059x; 1.2388x over previous
"""Trainium2 Bass kernel for windowed (sparse) attention — v3.

Module: LayerNorm -> overlapping 8x8 spatial windows (stride 6) over a
[2,2,128,128,256] image -> per-window 8-head attention over L=128 tokens
(t*8*8) -> output projection -> overlap-add with count normalization.

Strategy: 882 independent windows sharded over 8 cores (112 each, padded
to 896).  Host does im2win gather + overlap-add scatter; all model
compute runs on device.

v3 highlights (v1 = 1.82 ms, v2 = 598 us):
  - fp16 DRAM I/O; LN via bn_stats/bn_aggr
  - 1/sqrt(var+eps) via int-bit-trick + 2 Newton steps on the DVE,
    batched over 4-window groups -> the scalar engine runs ONLY
    Exp/Copy (one act table set, zero ACT_TABLE_LOAD thrash)
  - xn -> xnT via PE transpose (is_transpose matmul)
  - S matmuls K=128 with zero-padded head-major q ("qz", built by one
    gpsimd masked multiply); 4 heads per matmul (N=512, shared lhsT)
  - D (softmax denom) as 4 matmuls of N=256 via 3-dim rhs APs
  - softmax normalize via reciprocal_approx_fast + scalar_tensor_tensor
  - 11-deep software pipeline; PSUM fits exactly in 8 banks
"""

import functools
import math
from contextlib import ExitStack

import numpy as np

import concourse.bacc as bacc
import concourse.bass as bass
import concourse.mybir as mybir
import concourse.tile as tile
from concourse.bass import AP
from concourse.bass_utils import run_bass_kernel_spmd

# Force every ACT function this kernel uses (Exp, Ln, Copy) into the single
# combined table set `natural_log_exp_and_others` so the scalar engine never
# swaps activation tables (each swap costs ~2.7us).  Indices into
# act_info.json must be preserved, so we strip these functions from every
# other set rather than reordering.
_ORIG_GAT = bacc.get_activation_tables

def _patched_gat(arch):
    tabs = _ORIG_GAT(arch)
    strip = {mybir.ActivationFunctionType.from_pwp(n)
             for n in ("exp", "ln", "copy", "identity")}
    return {n: (fns if n == "natural_log_exp_and_others" else fns - strip)
            for n, fns in tabs.items()}

bacc.get_activation_tables = _patched_gat

# Problem constants (hardcoded per contract - kernel.py is self-contained).
B, T, H, W, C = 2, 2, 128, 128, 256
MID, HEADS = 256, 8
HD = MID // HEADS          # 32
PATCH, STEP = 8, 6         # window size / stride
NHW = 21                   # windows per axis: starts 0,6,...,120
NWIN = NHW * NHW * B       # 882 flat windows (n outer, b inner)
L = T * PATCH * PATCH      # 128 tokens per window
NCORES = 8
NW = 112                   # windows per core after padding to 896
EPS = 1e-6
MAGIC = 0x5F3759DF
F32, F16, I32 = mybir.dt.float32, mybir.dt.float16, mybir.dt.int32
AF = mybir.ActivationFunctionType
ALU = mybir.AluOpType


def _build_program(nw: int):
    nc = bacc.Bacc(
        "TRN2",
        target_bir_lowering=False,
        debug=False,
        enable_asserts=False,
        num_devices=NCORES,
    )
    xw = nc.dram_tensor("xw", [nw * 128, 256], F16, kind="ExternalInput").ap()
    wq = nc.dram_tensor("wq", [256, 256], F16, kind="ExternalInput").ap()
    wk = nc.dram_tensor("wk", [256, 256], F16, kind="ExternalInput").ap()
    wv = nc.dram_tensor("wv", [256, 256], F16, kind="ExternalInput").ap()
    wo = nc.dram_tensor("wo", [256, 256], F16, kind="ExternalInput").ap()
    ones1 = nc.dram_tensor("ones1", [128, 32], F16, kind="ExternalInput").ap()
    ident = nc.dram_tensor("ident", [128, 128], F16, kind="ExternalInput").ap()
    maskq = nc.dram_tensor("maskq", [128, 1024], F16, kind="ExternalInput").ap()
    zt = nc.dram_tensor("zt", [nw * 256, 128], F16, kind="ExternalOutput").ap()

    inv_sqrt_hd = 1.0 / math.sqrt(HD)
    GROUPS = nw // 4
    assert nw % 4 == 0

    with tile.TileContext(nc) as tc, ExitStack() as ctx:
        pw = ctx.enter_context(tc.tile_pool(name="wts", bufs=1))
        wq_s = [pw.tile([128, 256], F16, tag=f"wq{i}", name=f"wq{i}") for i in range(2)]
        wk_s = [pw.tile([128, 256], F16, tag=f"wk{i}", name=f"wk{i}") for i in range(2)]
        wv_s = [pw.tile([128, 256], F16, tag=f"wv{i}", name=f"wv{i}") for i in range(2)]
        wo_s = [pw.tile([128, 256], F16, tag=f"wo{i}", name=f"wo{i}") for i in range(2)]
        for i in range(2):
            nc.sync.dma_start(wq_s[i][:], wq[i * 128:(i + 1) * 128, :])
            nc.sync.dma_start(wk_s[i][:], wk[i * 128:(i + 1) * 128, :])
            nc.sync.dma_start(wv_s[i][:], wv[i * 128:(i + 1) * 128, :])
            nc.sync.dma_start(wo_s[i][:], wo[i * 128:(i + 1) * 128, :])
        ones_s = pw.tile([128, 32], F16, tag="ones1", name="ones1")
        nc.sync.dma_start(ones_s[:], ones1)
        id_s = pw.tile([128, 128], F16, tag="ident", name="ident")
        nc.sync.dma_start(id_s[:], ident)
        mq_s = pw.tile([128, 1024], F16, tag="maskq", name="maskq")
        nc.sync.dma_start(mq_s[:], maskq)
        eps_s = pw.tile([128, 1], F32, tag="eps", name="eps")
        nc.vector.memset(eps_s[:], EPS)

        # SBUF pools
        pxt = ctx.enter_context(tc.tile_pool(name="pxt", bufs=4))
        pxn = ctx.enter_context(tc.tile_pool(name="pxn", bufs=3))
        pxnt = ctx.enter_context(tc.tile_pool(name="pxnt", bufs=3))
        pqks = ctx.enter_context(tc.tile_pool(name="pqks", bufs=3))
        pqz = ctx.enter_context(tc.tile_pool(name="pqz", bufs=3))
        pvs = ctx.enter_context(tc.tile_pool(name="pvs", bufs=4))
        pes = ctx.enter_context(tc.tile_pool(name="pes", bufs=3))
        pos = ctx.enter_context(tc.tile_pool(name="pos", bufs=3))
        pzs = ctx.enter_context(tc.tile_pool(name="pzs", bufs=3))
        pst = ctx.enter_context(tc.tile_pool(name="pst", bufs=3))
        pch = ctx.enter_context(tc.tile_pool(name="pch", bufs=2))
        # PSUM pools: 1 + 1 + 1 + 2 + 3 = 8 banks
        ptp = ctx.enter_context(tc.tile_pool(name="ptp", bufs=1, space="PSUM"))
        pqk = ctx.enter_context(tc.tile_pool(name="pqk", bufs=1, space="PSUM"))
        pv = ctx.enter_context(tc.tile_pool(name="pv", bufs=1, space="PSUM"))
        psp = ctx.enter_context(tc.tile_pool(name="psp", bufs=1, space="PSUM"))
        pd = ctx.enter_context(tc.tile_pool(name="pd", bufs=2, space="PSUM"))
        pz = ctx.enter_context(tc.tile_pool(name="pz", bufs=1, space="PSUM"))

        # Pipeline state
        xt_g = {}
        bag_g, rs4_g = {}, {}
        xn_t, xnt_t, qks_t, qz_t, vs_t, es_t, os_t = {}, {}, {}, {}, {}, {}, {}
        dp_t, zp_t = {}, {}

        def load_group(g):
            t = pxt.tile([128, 1024], F16, tag="xt", name="xt")
            src = AP(xw.tensor, g * 4 * 128 * 256,
                     [[256, 128], [128 * 256, 4], [1, 256]])
            nc.sync.dma_start(t[:], src)
            xt_g[g] = t
            xt_g.pop(g - 3, None)

        load_group(0)

        for i in range(nw + 13):
            if (i + 2) % 4 == 0:
                g = (i + 2) // 4
                if g < GROUPS:
                    load_group(g)

            # ---- s0 (w=i): LN stats into group tile ----
            w = i
            if w < nw:
                g = w // 4
                if w % 4 == 0:
                    bag_g[g] = pst.tile([128, 8], F32, tag="bagg", name="bagg")
                xt = xt_g[g]
                xs = xt[:, (w % 4) * 256:(w % 4 + 1) * 256]
                bst = pst.tile([128, 6], F32, tag="bst", name="bst")
                nc.vector.bn_stats(bst[:], xs)
                nc.vector.bn_aggr(bag_g[g][:, 2 * (w % 4):2 * (w % 4) + 2], bst[:])

            # ---- group rsqrt: rs4 = Exp(-0.5*Ln(var4+eps)), 4 windows/op ----
            if i % 4 == 0 and i // 4 - 1 >= 0 and i // 4 - 1 < GROUPS:
                g = i // 4 - 1
                bag = bag_g[g]
                var4 = AP(bag[:].tensor, bag[:].offset + 1, [[8, 128], [2, 4]])
                lnv = pch.tile([128, 4], F32, tag="lnv", name="lnv")
                nc.scalar.activation(lnv[:], var4, AF.Ln, bias=eps_s[:])
                rs4 = pch.tile([128, 4], F32, tag="rs4", name="rs4")
                nc.scalar.activation(rs4[:], lnv[:], AF.Exp, scale=-0.5)
                rs4_g[g] = rs4

            # ---- s_apply (w=i-5): LN apply -> xn f16 ----
            w = i - 5
            if 0 <= w < nw:
                g = w // 4
                xt = xt_g[g]
                xs = xt[:, (w % 4) * 256:(w % 4 + 1) * 256]
                mu_w = pst.tile([128, 1], F32, tag="muw", name="muw")
                nc.vector.tensor_copy(mu_w[:],
                                      bag_g[g][:, 2 * (w % 4):2 * (w % 4) + 1])
                rs_w = pst.tile([128, 1], F32, tag="rsw", name="rsw")
                nc.vector.tensor_copy(rs_w[:], rs4_g[g][:, w % 4:w % 4 + 1])
                xn = pxn.tile([128, 256], F16, tag="xn", name="xn")
                nc.vector.tensor_scalar(
                    out=xn[:], in0=xs, scalar1=mu_w[:], scalar2=rs_w[:],
                    op0=ALU.subtract, op1=ALU.mult,
                )
                xn_t[w] = xn
                if w % 4 == 3:
                    bag_g.pop(g, None)
                    rs4_g.pop(g, None)

            # ---- s_T (w=i-6): PE transpose + evac ----
            w = i - 6
            if 0 <= w < nw:
                tp = ptp.tile([128, 1024], F16, tag="tp", name="tp")
                for kc in range(2):
                    nc.tensor.transpose(
                        tp[:, kc * 128:(kc + 1) * 128],
                        xn_t[w][:, kc * 128:(kc + 1) * 128], id_s[:])
                xnt = pxnt.tile([128, 256], F16, tag="xnt", name="xnt")
                nc.vector.tensor_copy(xnt[:], tp[:, 0:256])
                xnt_t[w] = xnt
                del xn_t[w]

            # ---- s_qkv (w=i-7): projections + evacs + qz ----
            w = i - 7
            if 0 <= w < nw:
                xnt = xnt_t[w]
                qkp = pqk.tile([128, 512], F32, tag="qk", name="qk")
                first = True
                for ws, base in ((wq_s, 0), (wk_s, 256)):
                    for mh in range(2):
                        for kc in range(2):
                            nc.tensor.matmul(
                                qkp[:, base + mh * 128:base + (mh + 1) * 128],
                                lhsT=ws[kc][:, mh * 128:(mh + 1) * 128],
                                rhs=xnt[:, kc * 128:(kc + 1) * 128],
                                start=first,
                                stop=(base == 256 and mh == 1 and kc == 1),
                            )
                            first = False
                vp = pv.tile([128, 512], F32, tag="v", name="v")
                for kc in range(2):
                    nc.tensor.matmul(
                        vp[:, 0:256], lhsT=xnt[:, kc * 128:(kc + 1) * 128],
                        rhs=wv_s[kc][:], start=(kc == 0), stop=(kc == 1),
                    )
                qks = pqks.tile([128, 512], F16, tag="qks", name="qks")
                nc.scalar.copy(qks[:], qkp[:])
                vs = pvs.tile([128, 256], F16, tag="vs", name="vs")
                nc.vector.tensor_copy(vs[:], vp[:, 0:256])
                qz = pqz.tile([128, 1024], F16, tag="qz", name="qz")
                qrep = AP(qks[:].tensor, qks[:].offset,
                          [[512, 128], [128, 2], [0, 4], [1, 128]])
                nc.gpsimd.tensor_tensor(out=qz[:], in0=qrep, in1=mq_s[:],
                                        op=ALU.mult)
                qks_t[w], qz_t[w], vs_t[w] = qks, qz, vs
                del xnt_t[w]

            # ---- s_S (w=i-8): S matmuls (4 heads per instr) + exp ----
            w = i - 8
            if 0 <= w < nw:
                qks, qz = qks_t[w], qz_t[w]
                sp = psp.tile([128, 1024], F32, tag="sp", name="sp")
                for mh in range(2):
                    nc.tensor.matmul(
                        sp[:, mh * 512:(mh + 1) * 512],
                        lhsT=qks[:, 256 + mh * 128:256 + (mh + 1) * 128],
                        rhs=qz[:, mh * 512:(mh + 1) * 512],
                        start=True, stop=True,
                    )
                es = pes.tile([128, 1024], F16, tag="es", name="es")
                nc.scalar.activation(es[:], sp[:], AF.Exp, scale=inv_sqrt_hd)
                es_t[w] = es
                del qks_t[w], qz_t[w]

            # ---- s_DOT (w=i-9): D (4x N=256) + OT (8x) matmuls ----
            w = i - 9
            if 0 <= w < nw:
                es, vs = es_t[w], vs_t[w]
                dp = pd.tile([128, 512], F32, tag="dp", name="dp")
                for j in range(4):
                    rhs = AP(es[:].tensor, es[:].offset + j * 128,
                             [[1024, 128], [512, 2], [1, 128]])
                    nc.tensor.matmul(
                        dp[32 * j:32 * j + 32, 0:256],
                        lhsT=ones_s[:], rhs=rhs,
                        start=True, stop=True, tile_position=(0, 32 * j),
                        skip_group_check=True,
                    )
                op_ = pd.tile([128, 512], F32, tag="dp", name="dp")
                for h in range(HEADS):
                    r, j = h // 4, h % 4
                    nc.tensor.matmul(
                        op_[32 * j:32 * j + 32, r * 128:(r + 1) * 128],
                        lhsT=vs[:, 32 * h:32 * h + 32],
                        rhs=es[:, h * 128:(h + 1) * 128],
                        start=(r == 0), stop=(r == 1), tile_position=(0, 32 * j),
                        skip_group_check=True,
                    )
                dp_t[w] = (dp, op_)
                del es_t[w], vs_t[w]

            # ---- s_norm (w=i-10): softmax normalize on DVE ----
            w = i - 10
            if 0 <= w < nw:
                dp, op_ = dp_t[w]
                dbs = pst.tile([128, 256], F32, tag="dbs", name="dbs")
                nc.vector.reciprocal_approx_fast(out=dbs[:], in_=dp[:, 0:256])
                os_ = pos.tile([128, 256], F16, tag="os", name="os")
                nc.vector.scalar_tensor_tensor(
                    out=os_[:], in0=op_[:, 0:256], scalar=1.0, in1=dbs[:],
                    op0=ALU.mult, op1=ALU.mult,
                )
                os_t[w] = os_
                del dp_t[w]

            # ---- s_Z (w=i-11): out projection ----
            w = i - 11
            if 0 <= w < nw:
                os_ = os_t[w]
                zp = pz.tile([128, 512], F32, tag="zp", name="zp")
                for coh in range(2):
                    for kc in range(2):
                        nc.tensor.matmul(
                            zp[:, coh * 128:(coh + 1) * 128],
                            lhsT=wo_s[kc][:, coh * 128:(coh + 1) * 128],
                            rhs=os_[:, kc * 128:(kc + 1) * 128],
                            start=(coh == 0 and kc == 0),
                            stop=(coh == 1 and kc == 1),
                        )
                zp_t[w] = zp
                del os_t[w]

            # ---- s_zs (w=i-12): evac + store ----
            w = i - 12
            if 0 <= w < nw:
                zp = zp_t[w]
                zs = pzs.tile([128, 256], F16, tag="zs", name="zs")
                nc.scalar.copy(zs[:], zp[:, 0:256])
                dst = AP(zt.tensor, w * 256 * 128,
                         [[128, 128], [128 * 128, 2], [1, 128]])
                nc.sync.dma_start(dst, zs[:])
                del zp_t[w]
    nc.compile()
    return nc


@functools.lru_cache(maxsize=2)
def _get_program(nw: int):
    return _build_program(nw)


def _im2win(x: np.ndarray) -> np.ndarray:
    """[B,T,H,W,C] -> [882,128,256] windows, flat order f = i_n*B + i_b."""
    s = x.strides
    xs = np.lib.stride_tricks.as_strided(
        x,
        shape=(B, T, NHW, PATCH, NHW, PATCH, C),
        strides=(s[0], s[1], STEP * s[2], s[2], STEP * s[3], s[3], s[4]),
    )
    w = xs.transpose(2, 4, 0, 1, 3, 5, 6)  # [iH,iW,b,t,p,q,c]
    return np.ascontiguousarray(w.reshape(NHW * NHW * B, L, C))


def _overlap_add(zwin: np.ndarray, bo: np.ndarray) -> np.ndarray:
    """[882,128,256] window outputs -> [B,T,H,W,C] with count-normalize + bo."""
    th = np.arange(NHW) * STEP
    z = zwin.reshape(B, NHW, NHW, T, PATCH, PATCH, MID)  # [b,iH,iW,t,p,q,c]
    acc = np.zeros((B, T, H, W, MID), np.float32)
    count = np.zeros((H, W), np.float32)
    for p in range(PATCH):
        rid = (th + p)[:, None]
        for q in range(PATCH):
            cid = (th + q)[None, :]
            acc[:, :, rid, cid, :] += z[:, :, :, :, p, q, :].transpose(0, 3, 1, 2, 4)
            count[rid, cid] += 1.0
    out = acc / count[None, None, :, :, None] + bo[None, None, None, None, :]
    return out


LAST_RESULT = None


def kernel(x, ln_g, ln_b, Wq, Wk, Wv, Wo, bo):
    x = np.asarray(x, np.float32)
    ln_g = np.asarray(ln_g, np.float32)
    ln_b = np.asarray(ln_b, np.float32)
    assert np.allclose(ln_b, 0.0), "kernel folds ln_g into weights; ln_b must be 0"
    # Fold LN gamma into the input side of Wq/Wk/Wv.
    wq_t = np.ascontiguousarray((np.asarray(Wq, np.float32) * ln_g).T.astype(np.float16))
    wk_t = np.ascontiguousarray((np.asarray(Wk, np.float32) * ln_g).T.astype(np.float16))
    wv_t = np.ascontiguousarray((np.asarray(Wv, np.float32) * ln_g).T.astype(np.float16))
    wo_t = np.ascontiguousarray(np.asarray(Wo, np.float32).T.astype(np.float16))
    ones1 = np.ones((128, 32), np.float16)
    ident = np.eye(128, dtype=np.float16)
    maskq = np.zeros((128, 1024), np.float16)
    for h in range(HEADS):
        j = h % 4
        maskq[32 * j:32 * j + 32, h * 128:(h + 1) * 128] = 1.0

    win = _im2win(x)                              # [882, 128, 256]
    pad = NCORES * NW - NWIN                      # 14
    winp = np.concatenate([win, np.zeros((pad, L, C), np.float32)], 0)
    shards = winp.reshape(NCORES, NW * L, C)

    nc = _get_program(NW)
    trace = bool(int(__import__("os").environ.get("KERNEL_TRACE", "0")))
    in_maps = []
    for i in range(NCORES):
        in_maps.append({
            "xw": np.ascontiguousarray(shards[i]).astype(np.float16),
            "wq": wq_t, "wk": wk_t, "wv": wv_t, "wo": wo_t,
            "ones1": ones1, "ident": ident, "maskq": maskq,
        })
    res = run_bass_kernel_spmd(nc, in_maps, core_ids=list(range(NCORES)),
                               trace=trace)
    global LAST_RESULT
    LAST_RESULT = res
    zts = [np.asarray(res.results[i]["zt"], np.float32).reshape(NW, 2, 128, 128)
           for i in range(NCORES)]
    # zt rows: w*256 + c_out, cols l  ->  Z_w[l, c] = zt[w, :, :, l]
    zall = np.concatenate(zts, 0)                 # [896, 2, 128, 128]
    zwin = zall.reshape(NCORES * NW, MID, L).transpose(0, 2, 1)[:NWIN]
    return _overlap_add(np.ascontiguousarray(zwin), np.asarray(bo, np.float32))
